# revision 3
# baseline (speedup 1.0000x reference)
"""Trainium2 Bass kernel for nn_CrossAttention_43061342110469.

Mathematical reduction: the reference's second einsum
    attn = einsum('bvhd,bhqk->bvhd', v, scores)
shares no contraction index with v, so it multiplies v elementwise by
S[b,h] = sum_{q,k} scores[b,h,q,k].  scores is a softmax over k, so every
row sums to 1 and S[b,h] == L == 2048 (exactly, even in fp32 — verified:
the fp32 reference computes S == 2048.0 bit-exactly, and the end-to-end
rel-err of this reduction vs the reference is ~5e-7).

Therefore:
    out = (x @ Wv + bv) * 2048 @ Wo + bo
        = ((2048*x) @ Wv + 2048*bv) @ Wo + bo

Kernel: row-shard the flattened [8192, 1024] x across 8 cores (1024 rows
each); each core runs two chained 1024x1024x1024 fp32 GEMMs:
    Phase T: DMA x rows, PE-transpose 128x128 tiles (scaling by 2048) into
             an SBUF x^T buffer (fp32 has no DMA-transpose path).
    Phase 1: v'^T[qkv, row] = Wv^T-tiles.T-free GEMM accumulating over d,
             plus a K=1 outer-product matmul adding 2048*bv.
    Phase 2: out[row, d_out] = v'^T-tiles as lhsT against Wo, plus a K=1
             outer-product matmul adding bo; DMA result tiles out.
q/k/softmax are numerically dead and not computed.
"""

import sys

import numpy as np

_REPO = "/opt/trn_rl_repo"
if _REPO not in sys.path:
    sys.path.insert(0, _REPO)

B, L, D = 4, 2048, 1024
NQKV = 1024  # QKV * H = 64 * 16
NCORES = 8
ROWS = B * L  # 8192
R = ROWS // NCORES  # 1024 rows per core
P = 128
NT = 512  # matmul free-dim tile (one PSUM bank of fp32)

_NC_CACHE = {}


def build_nc():
    """Build + compile the per-core Bass program (cached)."""
    if "nc" in _NC_CACHE:
        return _NC_CACHE["nc"]

    from contextlib import ExitStack

    import concourse.tile as tile
    from concourse import bacc, mybir
    from concourse._compat import get_trn_type
    from concourse.masks import make_identity

    f32 = mybir.dt.float32
    nc = bacc.Bacc(
        get_trn_type() or "TRN2",
        target_bir_lowering=False,
        debug=False,
        num_devices=NCORES,
    )

    x_nd = nc.dram_tensor("x", [R, D], f32, kind="ExternalInput").ap()
    wv_nd = nc.dram_tensor("wv", [D, NQKV], f32, kind="ExternalInput").ap()
    bv_nd = nc.dram_tensor("bv", [NQKV], f32, kind="ExternalInput").ap()
    wo_nd = nc.dram_tensor("wo", [NQKV, D], f32, kind="ExternalInput").ap()
    bo_nd = nc.dram_tensor("bo", [D], f32, kind="ExternalInput").ap()
    out_nd = nc.dram_tensor("out", [R, D], f32, kind="ExternalOutput").ap()

    KO = D // P  # 8 contraction tiles for GEMM1
    MQ = NQKV // P  # 8 qkv tiles (contraction tiles for GEMM2)
    RT = R // P  # 8 row tiles

    with tile.TileContext(nc) as tc, ExitStack() as ctx:
        const = ctx.enter_context(tc.tile_pool(name="const", bufs=1))
        big = ctx.enter_context(tc.tile_pool(name="big", bufs=1))
        xrow = ctx.enter_context(tc.tile_pool(name="xrow", bufs=2))
        tp = ctx.enter_context(tc.tile_pool(name="tpsum", bufs=2, space="PSUM"))
        mm = ctx.enter_context(tc.tile_pool(name="mmpsum", bufs=4, space="PSUM"))
        outp = ctx.enter_context(tc.tile_pool(name="outp", bufs=3))

        ident = const.tile([P, P], f32)
        make_identity(nc, ident)
        ones = const.tile([1, NT], f32)
        nc.vector.memset(ones[:], 1.0)
        bv2 = const.tile([1, NQKV], f32)
        nc.sync.dma_start(bv2[:], bv_nd[None, :])
        nc.vector.tensor_scalar_mul(bv2[:], bv2[:], 2048.0)
        bo_sb = const.tile([1, D], f32)
        nc.sync.dma_start(bo_sb[:], bo_nd[None, :])

        wv_sb = big.tile([P, KO, NQKV], f32)
        nc.sync.dma_start(wv_sb[:], wv_nd.rearrange("(ko p) n -> p ko n", p=P))
        wo_sb = big.tile([P, MQ, D], f32)
        nc.sync.dma_start(wo_sb[:], wo_nd.rearrange("(ko p) n -> p ko n", p=P))

        xT = big.tile([P, KO, R], f32)  # [d_inner, d_outer, row] = (2048*x)^T
        vT = big.tile([P, MQ, R], f32)  # [qkv_inner, qkv_outer, row]

        # Phase T: load x row-tiles, transpose 128x128 blocks on PE, scale 2048
        for r in range(RT):
            xt = xrow.tile([P, D], f32)
            nc.sync.dma_start(xt[:], x_nd[r * P : (r + 1) * P, :])
            for ko in range(KO):
                pt = tp.tile([P, P], f32)
                nc.tensor.transpose(pt[:], xt[:, ko * P : (ko + 1) * P], ident[:])
                nc.vector.tensor_scalar_mul(
                    xT[:, ko, r * P : (r + 1) * P], pt[:], 2048.0
                )

        # Phase 1: v'^T[qkv, row] = ((2048 x) @ Wv)^T + 2048*bv
        for m in range(MQ):
            for n in range(R // NT):
                ps = mm.tile([P, NT], f32)
                for ko in range(KO):
                    nc.tensor.matmul(
                        ps[:],
                        lhsT=wv_sb[:, ko, m * P : (m + 1) * P],
                        rhs=xT[:, ko, n * NT : (n + 1) * NT],
                        start=(ko == 0),
                        stop=False,
                    )
                # += 2048*bv[m-tile] broadcast along rows (K=1 outer product)
                nc.tensor.matmul(
                    ps[:],
                    lhsT=bv2[:, m * P : (m + 1) * P],
                    rhs=ones[:, :NT],
                    start=False,
                    stop=True,
                )
                nc.vector.tensor_copy(vT[:, m, n * NT : (n + 1) * NT], ps[:])

        # Phase 2: out[row, d_out] = v'^T.T @ Wo + bo
        for m in range(RT):
            for n in range(D // NT):
                ps = mm.tile([P, NT], f32)
                for ko in range(MQ):
                    nc.tensor.matmul(
                        ps[:],
                        lhsT=vT[:, ko, m * P : (m + 1) * P],
                        rhs=wo_sb[:, ko, n * NT : (n + 1) * NT],
                        start=(ko == 0),
                        stop=False,
                    )
                # += bo[n-tile] broadcast along rows (K=1 outer product)
                nc.tensor.matmul(
                    ps[:],
                    lhsT=ones[:, :P],
                    rhs=bo_sb[:, n * NT : (n + 1) * NT],
                    start=False,
                    stop=True,
                )
                ot = outp.tile([P, NT], f32)
                nc.vector.tensor_copy(ot[:], ps[:])
                nc.sync.dma_start(
                    out_nd[m * P : (m + 1) * P, n * NT : (n + 1) * NT], ot[:]
                )

    nc.compile()
    _NC_CACHE["nc"] = nc
    return nc


def make_in_maps(inputs):
    xf = np.ascontiguousarray(
        np.asarray(inputs["x"], dtype=np.float32).reshape(ROWS, D)
    )
    wv = np.ascontiguousarray(np.asarray(inputs["Wv"], dtype=np.float32))
    bv = np.ascontiguousarray(np.asarray(inputs["bv"], dtype=np.float32))
    wo = np.ascontiguousarray(np.asarray(inputs["Wo"], dtype=np.float32))
    bo = np.ascontiguousarray(np.asarray(inputs["bo"], dtype=np.float32))
    return [
        {
            "x": xf[c * R : (c + 1) * R],
            "wv": wv,
            "bv": bv,
            "wo": wo,
            "bo": bo,
        }
        for c in range(NCORES)
    ]


def kernel(**inputs) -> np.ndarray:
    from concourse.bass_utils import run_bass_kernel_spmd

    nc = build_nc()
    in_maps = make_in_maps(inputs)
    res = run_bass_kernel_spmd(nc, in_maps, list(range(NCORES)))
    out = np.concatenate(
        [res.results[c]["out"] for c in range(NCORES)], axis=0
    ).reshape(B, L, D)
    return np.ascontiguousarray(out.astype(np.float32, copy=False))


# revision 8
# speedup vs baseline: 2.1958x; 2.1958x over previous
"""Trainium2 Bass kernel for nn_CrossAttention_43061342110469.

Mathematical reduction: the reference's second einsum
    attn = einsum('bvhd,bhqk->bvhd', v, scores)
shares no contraction index with v, so it multiplies v elementwise by
S[b,h] = sum_{q,k} scores[b,h,q,k].  scores is a softmax over k, so every
row sums to 1 and S[b,h] == L == 2048 (exactly, even in fp32 — verified:
the fp32 reference computes S == 2048.0 bit-exactly, and the end-to-end
rel-err of this reduction vs the reference is ~5e-7).

Therefore:
    out = (x @ Wv + bv) * 2048 @ Wo + bo
        = ((2048*x) @ Wv + 2048*bv) @ Wo + bo

Kernel: row-shard the flattened [8192, 1024] x across 8 cores (1024 rows
each); each core runs two chained 1024x1024x1024 fp32 GEMMs:
    Phase T: DMA x rows, PE-transpose 128x128 tiles (scaling by 2048) into
             an SBUF x^T buffer (fp32 has no DMA-transpose path).
    Phase 1: v'^T[qkv, row] = Wv^T-tiles.T-free GEMM accumulating over d,
             plus a K=1 outer-product matmul adding 2048*bv.
    Phase 2: out[row, d_out] = v'^T-tiles as lhsT against Wo, plus a K=1
             outer-product matmul adding bo; DMA result tiles out.
q/k/softmax are numerically dead and not computed.
"""

import sys

import numpy as np

_REPO = "/opt/trn_rl_repo"
if _REPO not in sys.path:
    sys.path.insert(0, _REPO)

B, L, D = 4, 2048, 1024
NQKV = 1024  # QKV * H = 64 * 16
NCORES = 8
ROWS = B * L  # 8192
R = ROWS // NCORES  # 1024 rows per core
P = 128
NT = 512  # matmul free-dim tile (one PSUM bank of fp32)

# "float32" = exact fp32 matmul (2 HW passes, 4 cyc/row).
# "float32r" = single-pass fp32 matmul (1 cyc/row at N>=512), reduced
# internal mantissa; precision validated end-to-end against the reference.
MM_DTYPE = "float32r"

_NC_CACHE = {}


def build_nc():
    """Build + compile the per-core Bass program (cached)."""
    if "nc" in _NC_CACHE:
        return _NC_CACHE["nc"]

    from contextlib import ExitStack

    import concourse.tile as tile
    from concourse import bacc, mybir
    from concourse._compat import get_trn_type
    from concourse.masks import make_identity

    f32 = mybir.dt.float32
    mmdt = getattr(mybir.dt, MM_DTYPE)

    def mm(ps, lhsT, rhs, start, stop):
        nc.tensor.matmul(ps, lhsT=lhsT, rhs=rhs, start=start, stop=stop)

    nc = bacc.Bacc(
        get_trn_type() or "TRN2",
        target_bir_lowering=False,
        debug=False,
        num_devices=NCORES,
    )

    x_nd = nc.dram_tensor("x", [R, D], f32, kind="ExternalInput").ap()
    wv_nd = nc.dram_tensor("wv", [D, NQKV], f32, kind="ExternalInput").ap()
    bv_nd = nc.dram_tensor("bv", [NQKV], f32, kind="ExternalInput").ap()
    wo_nd = nc.dram_tensor("wo", [NQKV, D], f32, kind="ExternalInput").ap()
    bo_nd = nc.dram_tensor("bo", [D], f32, kind="ExternalInput").ap()
    out_nd = nc.dram_tensor("out", [R, D], f32, kind="ExternalOutput").ap()

    KO = D // P  # 8 contraction tiles for GEMM1
    MQ = NQKV // P  # 8 qkv tiles (contraction tiles for GEMM2)
    RT = R // P  # 8 row tiles

    with tile.TileContext(nc) as tc, ExitStack() as ctx:
        const = ctx.enter_context(tc.tile_pool(name="const", bufs=1))
        big = ctx.enter_context(tc.tile_pool(name="big", bufs=1))
        xrow = ctx.enter_context(tc.tile_pool(name="xrow", bufs=2))
        tp = ctx.enter_context(tc.tile_pool(name="tpsum", bufs=2, space="PSUM"))
        mmp = ctx.enter_context(tc.tile_pool(name="mmpsum", bufs=4, space="PSUM"))
        outp = ctx.enter_context(tc.tile_pool(name="outp", bufs=3))

        ident = const.tile([P, P], f32)
        make_identity(nc, ident)
        ones_f32 = const.tile([1, NT], f32)
        nc.vector.memset(ones_f32[:], 1.0)
        ones = const.tile([1, NT], mmdt)
        nc.vector.tensor_copy(ones[:], ones_f32[:])
        bv2 = const.tile([1, NQKV], mmdt)
        nc.sync.dma_start(bv2[:], bv_nd[None, :].bitcast(mmdt))
        nc.vector.tensor_scalar_mul(bv2[:], bv2[:], 2048.0)
        bo_sb = const.tile([1, D], mmdt)
        nc.sync.dma_start(bo_sb[:], bo_nd[None, :].bitcast(mmdt))

        wv_sb = big.tile([P, KO, NQKV], mmdt)
        nc.sync.dma_start(wv_sb[:], wv_nd.rearrange("(ko p) n -> p ko n", p=P).bitcast(mmdt))
        wo_sb = big.tile([P, MQ, D], mmdt)
        nc.sync.dma_start(wo_sb[:], wo_nd.rearrange("(ko p) n -> p ko n", p=P).bitcast(mmdt))

        xT = big.tile([P, KO, R], mmdt)  # [d_inner, d_outer, row] = (2048*x)^T
        vT = big.tile([P, MQ, R], mmdt)  # [qkv_inner, qkv_outer, row]

        # Phase T: load x row-tiles, transpose 128x128 blocks on PE, scale 2048
        for r in range(RT):
            xt = xrow.tile([P, D], f32)
            nc.sync.dma_start(xt[:], x_nd[r * P : (r + 1) * P, :])
            for ko in range(KO):
                pt = tp.tile([P, P], f32)
                nc.tensor.transpose(pt[:], xt[:, ko * P : (ko + 1) * P], ident[:])
                nc.vector.tensor_scalar_mul(
                    xT[:, ko, r * P : (r + 1) * P], pt[:], 2048.0
                )

        # Phase 1: v'^T[qkv, row] = ((2048 x) @ Wv)^T + 2048*bv
        for m in range(MQ):
            for n in range(R // NT):
                ps = mmp.tile([P, NT], f32)
                for ko in range(KO):
                    mm(
                        ps[:],
                        wv_sb[:, ko, m * P : (m + 1) * P],
                        xT[:, ko, n * NT : (n + 1) * NT],
                        start=(ko == 0),
                        stop=False,
                    )
                # += 2048*bv[m-tile] broadcast along rows (K=1 outer product)
                mm(ps[:], bv2[:, m * P : (m + 1) * P], ones[:, :NT], False, True)
                nc.vector.tensor_copy(vT[:, m, n * NT : (n + 1) * NT], ps[:])

        # Phase 2: out[row, d_out] = v'^T.T @ Wo + bo
        for m in range(RT):
            for n in range(D // NT):
                ps = mmp.tile([P, NT], f32)
                for ko in range(MQ):
                    mm(
                        ps[:],
                        vT[:, ko, m * P : (m + 1) * P],
                        wo_sb[:, ko, n * NT : (n + 1) * NT],
                        start=(ko == 0),
                        stop=False,
                    )
                # += bo[n-tile] broadcast along rows (K=1 outer product)
                mm(ps[:], ones[:, :P], bo_sb[:, n * NT : (n + 1) * NT], False, True)
                ot = outp.tile([P, NT], f32)
                nc.vector.tensor_copy(ot[:], ps[:])
                nc.sync.dma_start(
                    out_nd[m * P : (m + 1) * P, n * NT : (n + 1) * NT], ot[:]
                )

    nc.compile()
    _NC_CACHE["nc"] = nc
    return nc


def make_in_maps(inputs):
    xf = np.ascontiguousarray(
        np.asarray(inputs["x"], dtype=np.float32).reshape(ROWS, D)
    )
    wv = np.ascontiguousarray(np.asarray(inputs["Wv"], dtype=np.float32))
    bv = np.ascontiguousarray(np.asarray(inputs["bv"], dtype=np.float32))
    wo = np.ascontiguousarray(np.asarray(inputs["Wo"], dtype=np.float32))
    bo = np.ascontiguousarray(np.asarray(inputs["bo"], dtype=np.float32))
    return [
        {
            "x": xf[c * R : (c + 1) * R],
            "wv": wv,
            "bv": bv,
            "wo": wo,
            "bo": bo,
        }
        for c in range(NCORES)
    ]


def kernel(**inputs) -> np.ndarray:
    from concourse.bass_utils import run_bass_kernel_spmd

    nc = build_nc()
    in_maps = make_in_maps(inputs)
    res = run_bass_kernel_spmd(nc, in_maps, list(range(NCORES)))
    out = np.concatenate(
        [res.results[c]["out"] for c in range(NCORES)], axis=0
    ).reshape(B, L, D)
    return np.ascontiguousarray(out.astype(np.float32, copy=False))


# revision 10
# speedup vs baseline: 2.4226x; 1.1033x over previous
"""Trainium2 Bass kernel for nn_CrossAttention_43061342110469.

Mathematical reduction: the reference's second einsum
    attn = einsum('bvhd,bhqk->bvhd', v, scores)
shares no contraction index with v, so it multiplies v elementwise by
S[b,h] = sum_{q,k} scores[b,h,q,k].  scores is a softmax over k, so every
row sums to 1 and S[b,h] == L == 2048 (exactly, even in fp32 — verified:
the fp32 reference computes S == 2048.0 bit-exactly, and the end-to-end
rel-err of this reduction vs the reference is ~5e-7).

Therefore:
    out = (x @ Wv + bv) * 2048 @ Wo + bo
        = ((2048*x) @ Wv + 2048*bv) @ Wo + bo

Kernel: row-shard the flattened [8192, 1024] x across 8 cores (1024 rows
each); each core runs two chained 1024x1024x1024 fp32 GEMMs:
    Phase T: DMA x rows, PE-transpose 128x128 tiles (scaling by 2048) into
             an SBUF x^T buffer (fp32 has no DMA-transpose path).
    Phase 1: v'^T[qkv, row] = Wv^T-tiles.T-free GEMM accumulating over d,
             plus a K=1 outer-product matmul adding 2048*bv.
    Phase 2: out[row, d_out] = v'^T-tiles as lhsT against Wo, plus a K=1
             outer-product matmul adding bo; DMA result tiles out.
q/k/softmax are numerically dead and not computed.
"""

import sys

import numpy as np

_REPO = "/opt/trn_rl_repo"
if _REPO not in sys.path:
    sys.path.insert(0, _REPO)

B, L, D = 4, 2048, 1024
NQKV = 1024  # QKV * H = 64 * 16
NCORES = 8
ROWS = B * L  # 8192
R = ROWS // NCORES  # 1024 rows per core
P = 128
NT = 512  # matmul free-dim tile (one PSUM bank of fp32)

# "float32" = exact fp32 matmul (2 HW passes, 4 cyc/row).
# "float32r" = single-pass fp32 matmul (1 cyc/row at N>=512), reduced
# internal mantissa; precision validated end-to-end against the reference.
MM_DTYPE = "float32r"

_NC_CACHE = {}


def build_nc():
    """Build + compile the per-core Bass program (cached)."""
    if "nc" in _NC_CACHE:
        return _NC_CACHE["nc"]

    from contextlib import ExitStack

    import concourse.tile as tile
    from concourse import bacc, mybir
    from concourse._compat import get_trn_type
    from concourse.masks import make_identity

    f32 = mybir.dt.float32
    mmdt = getattr(mybir.dt, MM_DTYPE)

    def mm(ps, lhsT, rhs, start, stop):
        nc.tensor.matmul(ps, lhsT=lhsT, rhs=rhs, start=start, stop=stop)

    nc = bacc.Bacc(
        get_trn_type() or "TRN2",
        target_bir_lowering=False,
        debug=False,
        num_devices=NCORES,
    )

    x_nd = nc.dram_tensor("x", [R, D], f32, kind="ExternalInput").ap()
    wv_nd = nc.dram_tensor("wv", [D, NQKV], f32, kind="ExternalInput").ap()
    bv_nd = nc.dram_tensor("bv", [NQKV], f32, kind="ExternalInput").ap()
    wo_nd = nc.dram_tensor("wo", [NQKV, D], f32, kind="ExternalInput").ap()
    bo_nd = nc.dram_tensor("bo", [D], f32, kind="ExternalInput").ap()
    out_nd = nc.dram_tensor("out", [R, D], f32, kind="ExternalOutput").ap()

    KO = D // P  # 8 contraction tiles for GEMM1
    MQ = NQKV // P  # 8 qkv tiles (contraction tiles for GEMM2)
    RT = R // P  # 8 row tiles

    with tile.TileContext(nc) as tc, ExitStack() as ctx:
        const = ctx.enter_context(tc.tile_pool(name="const", bufs=1))
        big = ctx.enter_context(tc.tile_pool(name="big", bufs=1))
        xrow = ctx.enter_context(tc.tile_pool(name="xrow", bufs=4))
        tp = ctx.enter_context(tc.tile_pool(name="tpsum", bufs=2, space="PSUM"))
        mmp = ctx.enter_context(tc.tile_pool(name="mmpsum", bufs=6, space="PSUM"))
        outp = ctx.enter_context(tc.tile_pool(name="outp", bufs=3))

        ident = const.tile([P, P], f32)
        make_identity(nc, ident)
        ones_f32 = const.tile([1, NT], f32)
        nc.vector.memset(ones_f32[:], 1.0)
        ones = const.tile([1, NT], mmdt)
        nc.vector.tensor_copy(ones[:], ones_f32[:])
        bv2 = const.tile([1, NQKV], mmdt)
        nc.sync.dma_start(bv2[:], bv_nd[None, :].bitcast(mmdt))
        nc.vector.tensor_scalar_mul(bv2[:], bv2[:], 2048.0)
        bo_sb = const.tile([1, D], mmdt)
        nc.sync.dma_start(bo_sb[:], bo_nd[None, :].bitcast(mmdt))

        wv_sb = big.tile([P, KO, NQKV], mmdt)
        wo_sb = big.tile([P, MQ, D], mmdt)
        xT = big.tile([P, KO, R], mmdt)  # [d_inner, d_outer, row] = (2048*x)^T
        vT = big.tile([P, MQ, R], mmdt)  # [qkv_inner, qkv_outer, row]

        # x row-tiles first on the sync DMA queue: transposes (PE) can start
        # ~2us in instead of waiting behind 8 MiB of weights.
        xts = []
        for r in range(RT):
            xt = xrow.tile([P, D], f32)
            nc.sync.dma_start(xt[:], x_nd[r * P : (r + 1) * P, :])
            xts.append(xt)
        # Weights stream on the gpsimd DMA queue in per-ko chunks, in
        # parallel with the x loads and the transpose phase.
        wv_r = wv_nd.rearrange("(ko p) n -> p ko n", p=P).bitcast(mmdt)
        wo_r = wo_nd.rearrange("(ko p) n -> p ko n", p=P).bitcast(mmdt)
        for ko in range(KO):
            nc.gpsimd.dma_start(wv_sb[:, ko], wv_r[:, ko])
        for ko in range(MQ):
            nc.gpsimd.dma_start(wo_sb[:, ko], wo_r[:, ko])

        # Phase T: transpose 128x128 blocks on PE, scale by 2048
        for r in range(RT):
            xt = xts[r]
            for ko in range(KO):
                pt = tp.tile([P, P], f32)
                nc.tensor.transpose(pt[:], xt[:, ko * P : (ko + 1) * P], ident[:])
                nc.vector.tensor_scalar_mul(
                    xT[:, ko, r * P : (r + 1) * P], pt[:], 2048.0
                )

        # Phase 1: v'^T[qkv, row] = ((2048 x) @ Wv)^T + 2048*bv
        for m in range(MQ):
            for n in range(R // NT):
                ps = mmp.tile([P, NT], f32)
                for ko in range(KO):
                    mm(
                        ps[:],
                        wv_sb[:, ko, m * P : (m + 1) * P],
                        xT[:, ko, n * NT : (n + 1) * NT],
                        start=(ko == 0),
                        stop=False,
                    )
                # += 2048*bv[m-tile] broadcast along rows (K=1 outer product)
                mm(ps[:], bv2[:, m * P : (m + 1) * P], ones[:, :NT], False, True)
                nc.vector.tensor_copy(vT[:, m, n * NT : (n + 1) * NT], ps[:])

        # Phase 2: out[row, d_out] = v'^T.T @ Wo + bo
        for m in range(RT):
            for n in range(D // NT):
                ps = mmp.tile([P, NT], f32)
                for ko in range(MQ):
                    mm(
                        ps[:],
                        vT[:, ko, m * P : (m + 1) * P],
                        wo_sb[:, ko, n * NT : (n + 1) * NT],
                        start=(ko == 0),
                        stop=False,
                    )
                # += bo[n-tile] broadcast along rows (K=1 outer product)
                mm(ps[:], ones[:, :P], bo_sb[:, n * NT : (n + 1) * NT], False, True)
                ot = outp.tile([P, NT], f32)
                nc.vector.tensor_copy(ot[:], ps[:])
                nc.sync.dma_start(
                    out_nd[m * P : (m + 1) * P, n * NT : (n + 1) * NT], ot[:]
                )

    nc.compile()
    _NC_CACHE["nc"] = nc
    return nc


def make_in_maps(inputs):
    xf = np.ascontiguousarray(
        np.asarray(inputs["x"], dtype=np.float32).reshape(ROWS, D)
    )
    wv = np.ascontiguousarray(np.asarray(inputs["Wv"], dtype=np.float32))
    bv = np.ascontiguousarray(np.asarray(inputs["bv"], dtype=np.float32))
    wo = np.ascontiguousarray(np.asarray(inputs["Wo"], dtype=np.float32))
    bo = np.ascontiguousarray(np.asarray(inputs["bo"], dtype=np.float32))
    return [
        {
            "x": xf[c * R : (c + 1) * R],
            "wv": wv,
            "bv": bv,
            "wo": wo,
            "bo": bo,
        }
        for c in range(NCORES)
    ]


def kernel(**inputs) -> np.ndarray:
    from concourse.bass_utils import run_bass_kernel_spmd

    nc = build_nc()
    in_maps = make_in_maps(inputs)
    res = run_bass_kernel_spmd(nc, in_maps, list(range(NCORES)))
    out = np.concatenate(
        [res.results[c]["out"] for c in range(NCORES)], axis=0
    ).reshape(B, L, D)
    return np.ascontiguousarray(out.astype(np.float32, copy=False))


# revision 15
# speedup vs baseline: 2.6434x; 1.0911x over previous
"""Trainium2 Bass kernel for nn_CrossAttention_43061342110469.

Mathematical reduction: the reference's second einsum
    attn = einsum('bvhd,bhqk->bvhd', v, scores)
shares no contraction index with v, so it multiplies v elementwise by
S[b,h] = sum_{q,k} scores[b,h,q,k].  scores is a softmax over k, so every
row sums to 1 and S[b,h] == L == 2048 (exactly, even in fp32 — verified:
the fp32 reference computes S == 2048.0 bit-exactly, and the end-to-end
rel-err of this reduction vs the reference is ~5e-7).

Therefore:
    out = (x @ Wv + bv) * 2048 @ Wo + bo
        = ((2048*x) @ Wv + 2048*bv) @ Wo + bo

Kernel: row-shard the flattened [8192, 1024] x across 8 cores (1024 rows
each); each core runs two chained 1024x1024x1024 fp32 GEMMs:
    Phase T: DMA x rows, PE-transpose 128x128 tiles (scaling by 2048) into
             an SBUF x^T buffer (fp32 has no DMA-transpose path).
    Phase 1: v'^T[qkv, row] = Wv^T-tiles.T-free GEMM accumulating over d,
             plus a K=1 outer-product matmul adding 2048*bv.
    Phase 2: out[row, d_out] = v'^T-tiles as lhsT against Wo, plus a K=1
             outer-product matmul adding bo; DMA result tiles out.
q/k/softmax are numerically dead and not computed.
"""

import sys

import numpy as np

_REPO = "/opt/trn_rl_repo"
if _REPO not in sys.path:
    sys.path.insert(0, _REPO)

B, L, D = 4, 2048, 1024
NQKV = 1024  # QKV * H = 64 * 16
NCORES = 8
ROWS = B * L  # 8192
R = ROWS // NCORES  # 1024 rows per core
P = 128
NT = 512  # matmul free-dim tile (one PSUM bank of fp32)

# "float32" = exact fp32 matmul (2 HW passes, 4 cyc/row).
# "float32r" = single-pass fp32 matmul (1 cyc/row at N>=512), reduced
# internal mantissa; precision validated end-to-end against the reference.
MM_DTYPE = "float32r"

_NC_CACHE = {}


def build_nc():
    """Build + compile the per-core Bass program (cached)."""
    if "nc" in _NC_CACHE:
        return _NC_CACHE["nc"]

    from contextlib import ExitStack

    import concourse.tile as tile
    from concourse import bacc, mybir
    from concourse._compat import get_trn_type
    from concourse.masks import make_identity

    f32 = mybir.dt.float32
    mmdt = getattr(mybir.dt, MM_DTYPE)

    def mm(ps, lhsT, rhs, start, stop):
        nc.tensor.matmul(ps, lhsT=lhsT, rhs=rhs, start=start, stop=stop)

    nc = bacc.Bacc(
        get_trn_type() or "TRN2",
        target_bir_lowering=False,
        debug=False,
        num_devices=NCORES,
    )

    x_nd = nc.dram_tensor("x", [R, D], f32, kind="ExternalInput").ap()
    wv_nd = nc.dram_tensor("wv", [D, NQKV], f32, kind="ExternalInput").ap()
    bv_nd = nc.dram_tensor("bv", [NQKV], f32, kind="ExternalInput").ap()
    wo_nd = nc.dram_tensor("wo", [NQKV, D], f32, kind="ExternalInput").ap()
    bo_nd = nc.dram_tensor("bo", [D], f32, kind="ExternalInput").ap()
    out_nd = nc.dram_tensor("out", [R, D], f32, kind="ExternalOutput").ap()

    KO = D // P  # 8 contraction tiles for GEMM1
    MQ = NQKV // P  # 8 qkv tiles (contraction tiles for GEMM2)
    RT = R // P  # 8 row tiles

    with tile.TileContext(nc) as tc, ExitStack() as ctx:
        const = ctx.enter_context(tc.tile_pool(name="const", bufs=1))
        big = ctx.enter_context(tc.tile_pool(name="big", bufs=1))
        xrow = ctx.enter_context(tc.tile_pool(name="xrow", bufs=4))
        tp = ctx.enter_context(tc.tile_pool(name="tpsum", bufs=2, space="PSUM"))
        mmp = ctx.enter_context(tc.tile_pool(name="mmpsum", bufs=6, space="PSUM"))
        outp = ctx.enter_context(tc.tile_pool(name="outp", bufs=3))

        # --- PE warmup: ~5us of dummy matmuls so the HAM clock-gate opens
        # (K=8/8 @ 2.4GHz) before the real transposes arrive.
        warm = const.tile([P, P], f32)
        nc.vector.memset(warm[:], 1.0)
        wps = tp.tile([P, P], f32, tag="t")
        for _ in range(12):
            nc.tensor.matmul(wps[:], lhsT=warm[:], rhs=warm[:], start=True, stop=True)

        ident = const.tile([P, P], f32)
        make_identity(nc, ident)
        ident_r = const.tile([P, P], mmdt)
        nc.vector.tensor_copy(ident_r[:], ident[:])

        # biases: bv2[p, o] = 2048*bv[o*128+p] (per-partition scalar for GEMM1
        # copyback); bo_rep = bo broadcast across partitions (free-dim bias for
        # GEMM2 copyback).  Both on the vector DMA queue, early.
        bv2 = const.tile([P, NQKV // P], f32)
        nc.scalar.dma_start(bv2[:], bv_nd.rearrange("(o p) -> p o", p=P))
        nc.vector.tensor_scalar_mul(bv2[:], bv2[:], 2048.0)
        bo_rep = const.tile([P, D], f32)
        nc.scalar.dma_start(bo_rep[:], bo_nd[None, :].to_broadcast((P, D)))

        wv_sb = big.tile([P, KO, NQKV], mmdt)
        wo_sb = big.tile([P, MQ, D], mmdt)
        xT = big.tile([P, KO, R], mmdt)  # [d_inner, d_outer, row] = (2048*x)^T
        vT = big.tile([P, MQ, R], mmdt)  # [qkv_inner, qkv_outer, row]

        # x row-tiles on the sync DMA queue: transposes (PE) can start ~10us in.
        xts = []
        for r in range(RT):
            xt = xrow.tile([P, D], mmdt)
            nc.sync.dma_start(xt[:], x_nd[r * P : (r + 1) * P, :].bitcast(mmdt))
            xts.append(xt)
        # Wv chunks on the scalar queue (idle), Wo chunks on gpsimd — all three
        # weight/x streams issue in parallel with the transpose phase.
        wv_r = wv_nd.rearrange("(ko p) n -> p ko n", p=P).bitcast(mmdt)
        wo_r = wo_nd.rearrange("(ko p) n -> p ko n", p=P).bitcast(mmdt)
        for ko in range(KO):
            nc.scalar.dma_start(wv_sb[:, ko], wv_r[:, ko])
        for ko in range(MQ):
            nc.gpsimd.dma_start(wo_sb[:, ko], wo_r[:, ko])

        # Phase T: transpose 128x128 blocks on PE (f32r path), scale by 2048
        for r in range(RT):
            xt = xts[r]
            for ko in range(KO):
                pt = tp.tile([P, P], mmdt, tag="t")
                nc.tensor.transpose(
                    pt[:], xt[:, ko * P : (ko + 1) * P], ident_r[:]
                )
                nc.vector.tensor_scalar_mul(
                    xT[:, ko, r * P : (r + 1) * P], pt[:], 2048.0
                )

        # Phase 1: v'^T[qkv, row] = ((2048 x) @ Wv)^T; bias fused in copyback
        for m in range(MQ):
            for n in range(R // NT):
                ps = mmp.tile([P, NT], f32)
                for ko in range(KO):
                    mm(
                        ps[:],
                        wv_sb[:, ko, m * P : (m + 1) * P],
                        xT[:, ko, n * NT : (n + 1) * NT],
                        start=(ko == 0),
                        stop=(ko == KO - 1),
                    )
                # vT = ps + 2048*bv (per-partition scalar), rounded to f32r
                nc.vector.tensor_scalar_add(
                    vT[:, m, n * NT : (n + 1) * NT], ps[:], bv2[:, m : m + 1]
                )

        # Phase 2: out[row, d_out] = v'^T.T @ Wo; bo fused in copyback
        for m in range(RT):
            for n in range(D // NT):
                ps = mmp.tile([P, NT], f32)
                for ko in range(MQ):
                    mm(
                        ps[:],
                        vT[:, ko, m * P : (m + 1) * P],
                        wo_sb[:, ko, n * NT : (n + 1) * NT],
                        start=(ko == 0),
                        stop=(ko == MQ - 1),
                    )
                ot = outp.tile([P, NT], f32)
                nc.vector.tensor_tensor(
                    ot[:], ps[:], bo_rep[:, n * NT : (n + 1) * NT], mybir.AluOpType.add
                )
                nc.sync.dma_start(
                    out_nd[m * P : (m + 1) * P, n * NT : (n + 1) * NT], ot[:]
                )

    nc.compile()
    _NC_CACHE["nc"] = nc
    return nc


def make_in_maps(inputs):
    xf = np.ascontiguousarray(
        np.asarray(inputs["x"], dtype=np.float32).reshape(ROWS, D)
    )
    wv = np.ascontiguousarray(np.asarray(inputs["Wv"], dtype=np.float32))
    bv = np.ascontiguousarray(np.asarray(inputs["bv"], dtype=np.float32))
    wo = np.ascontiguousarray(np.asarray(inputs["Wo"], dtype=np.float32))
    bo = np.ascontiguousarray(np.asarray(inputs["bo"], dtype=np.float32))
    return [
        {
            "x": xf[c * R : (c + 1) * R],
            "wv": wv,
            "bv": bv,
            "wo": wo,
            "bo": bo,
        }
        for c in range(NCORES)
    ]


def kernel(**inputs) -> np.ndarray:
    from concourse.bass_utils import run_bass_kernel_spmd

    nc = build_nc()
    in_maps = make_in_maps(inputs)
    res = run_bass_kernel_spmd(nc, in_maps, list(range(NCORES)))
    out = np.concatenate(
        [res.results[c]["out"] for c in range(NCORES)], axis=0
    ).reshape(B, L, D)
    return np.ascontiguousarray(out.astype(np.float32, copy=False))


# revision 17
# speedup vs baseline: 3.1049x; 1.1746x over previous
"""Trainium2 Bass kernel for nn_CrossAttention_43061342110469.

Mathematical reduction: the reference's second einsum
    attn = einsum('bvhd,bhqk->bvhd', v, scores)
shares no contraction index with v, so it multiplies v elementwise by
S[b,h] = sum_{q,k} scores[b,h,q,k].  scores is a softmax over k, so every
row sums to 1 and S[b,h] == L == 2048 (exactly, even in fp32 — verified:
the fp32 reference computes S == 2048.0 bit-exactly, and the end-to-end
rel-err of this reduction vs the reference is ~5e-7).

Therefore:
    out = (x @ Wv + bv) * 2048 @ Wo + bo
        = ((2048*x) @ Wv + 2048*bv) @ Wo + bo

Kernel: row-shard the flattened [8192, 1024] x across 8 cores (1024 rows
each); each core runs two chained 1024x1024x1024 fp32 GEMMs:
    Phase T: DMA x rows, PE-transpose 128x128 tiles (scaling by 2048) into
             an SBUF x^T buffer (fp32 has no DMA-transpose path).
    Phase 1: v'^T[qkv, row] = Wv^T-tiles.T-free GEMM accumulating over d,
             plus a K=1 outer-product matmul adding 2048*bv.
    Phase 2: out[row, d_out] = v'^T-tiles as lhsT against Wo, plus a K=1
             outer-product matmul adding bo; DMA result tiles out.
q/k/softmax are numerically dead and not computed.
"""

import sys

import numpy as np

_REPO = "/opt/trn_rl_repo"
if _REPO not in sys.path:
    sys.path.insert(0, _REPO)

B, L, D = 4, 2048, 1024
NQKV = 1024  # QKV * H = 64 * 16
NCORES = 8
ROWS = B * L  # 8192
R = ROWS // NCORES  # 1024 rows per core
P = 128
NT = 512  # matmul free-dim tile (one PSUM bank of fp32)

# "float32" = exact fp32 matmul (2 HW passes, 4 cyc/row).
# "float32r" = single-pass fp32 matmul (1 cyc/row at N>=512), reduced
# internal mantissa; precision validated end-to-end against the reference.
MM_DTYPE = "float32r"

_NC_CACHE = {}


def build_nc():
    """Build + compile the per-core Bass program (cached)."""
    if "nc" in _NC_CACHE:
        return _NC_CACHE["nc"]

    from contextlib import ExitStack

    import concourse.tile as tile
    from concourse import bacc, mybir
    from concourse._compat import get_trn_type
    from concourse.masks import make_identity

    f32 = mybir.dt.float32
    mmdt = getattr(mybir.dt, MM_DTYPE)

    def mm(ps, lhsT, rhs, start, stop):
        nc.tensor.matmul(ps, lhsT=lhsT, rhs=rhs, start=start, stop=stop)

    nc = bacc.Bacc(
        get_trn_type() or "TRN2",
        target_bir_lowering=False,
        debug=False,
        num_devices=NCORES,
    )

    x_nd = nc.dram_tensor("x", [R, D], f32, kind="ExternalInput").ap()
    wv_nd = nc.dram_tensor("wv", [D, NQKV], f32, kind="ExternalInput").ap()
    bv_nd = nc.dram_tensor("bv", [NQKV], f32, kind="ExternalInput").ap()
    wo_nd = nc.dram_tensor("wo", [NQKV, D], f32, kind="ExternalInput").ap()
    bo_nd = nc.dram_tensor("bo", [D], f32, kind="ExternalInput").ap()
    out_nd = nc.dram_tensor("out", [R, D], f32, kind="ExternalOutput").ap()

    KO = D // P  # 8 contraction tiles for GEMM1
    MQ = NQKV // P  # 8 qkv tiles (contraction tiles for GEMM2)
    RT = R // P  # 8 row tiles

    with tile.TileContext(nc) as tc, ExitStack() as ctx:
        const = ctx.enter_context(tc.tile_pool(name="const", bufs=1))
        big = ctx.enter_context(tc.tile_pool(name="big", bufs=1))
        xrow = ctx.enter_context(tc.tile_pool(name="xrow", bufs=4))
        tp = ctx.enter_context(tc.tile_pool(name="tpsum", bufs=2, space="PSUM"))
        mmp = ctx.enter_context(tc.tile_pool(name="mmpsum", bufs=3, space="PSUM"))
        outp = ctx.enter_context(tc.tile_pool(name="outp", bufs=3))

        # --- PE warmup: ~5us of dummy matmuls so the HAM clock-gate opens
        # (K=8/8 @ 2.4GHz) before the real transposes arrive.
        warm = const.tile([P, P], f32)
        nc.vector.memset(warm[:], 1.0)
        wps = tp.tile([P, P], f32, tag="t")
        for _ in range(12):
            nc.tensor.matmul(wps[:], lhsT=warm[:], rhs=warm[:], start=True, stop=True)

        ident = const.tile([P, P], f32)
        make_identity(nc, ident)
        ident_r = const.tile([P, P], mmdt)
        nc.vector.tensor_copy(ident_r[:], ident[:])

        # biases: bv2[p, o] = 2048*bv[o*128+p] (per-partition scalar for GEMM1
        # copyback); bo_rep = bo broadcast across partitions (free-dim bias for
        # GEMM2 copyback).  Both on the vector DMA queue, early.
        bv2 = const.tile([P, NQKV // P], f32)
        nc.scalar.dma_start(bv2[:], bv_nd.rearrange("(o p) -> p o", p=P))
        nc.vector.tensor_scalar_mul(bv2[:], bv2[:], 2048.0)
        bo_rep = const.tile([P, D], f32)
        nc.scalar.dma_start(bo_rep[:], bo_nd[None, :].to_broadcast((P, D)))

        wv_sb = big.tile([P, KO, NQKV], mmdt)
        wo_sb = big.tile([P, MQ, D], mmdt)
        xT = big.tile([P, KO, R], mmdt)  # [d_inner, d_outer, row] = (2048*x)^T
        vT = big.tile([P, MQ, R], mmdt)  # [qkv_inner, qkv_outer, row]

        # x rows + Wv chunks interleaved round-robin across the three DMA
        # issue queues (sync / scalar / gpsimd): each dma_start is packet-
        # sharded over all 16 DMA engines, so concurrency across queues is
        # what buys aggregate HBM bandwidth (~370 GB/s).  x feeds the PE
        # transposes starting ~10us; Wv must be complete by ~22us for GEMM1;
        # Wo (gpsimd, last) is only needed by GEMM2 at ~50us.
        wv_r = wv_nd.rearrange("(ko p) n -> p ko n", p=P).bitcast(mmdt)
        wo_r = wo_nd.rearrange("(ko p) n -> p ko n", p=P).bitcast(mmdt)
        qs = [nc.sync, nc.scalar, nc.gpsimd]
        jobs = []
        for r in range(RT):
            xt = xrow.tile([P, D], mmdt, tag="xt", name=f"xt{r}")
            jobs.append(("x", xt, x_nd[r * P : (r + 1) * P, :].bitcast(mmdt)))
        for ko in range(KO):
            jobs.append(("wv", wv_sb[:, ko], wv_r[:, ko]))
        xts = [None] * RT
        xi = 0
        for i, (kind, dst, srcap) in enumerate(jobs):
            q = qs[i % 3]
            if kind == "x":
                q.dma_start(dst[:], srcap)
                xts[xi] = dst
                xi += 1
            else:
                q.dma_start(dst, srcap)
        for ko in range(MQ):
            nc.gpsimd.dma_start(wo_sb[:, ko], wo_r[:, ko])

        # Phase T: transpose 128x128 blocks on PE (f32r path), scale by 2048
        def transpose_rows(rs):
            for r in rs:
                xt = xts[r]
                for ko in range(KO):
                    pt = tp.tile([P, P], mmdt, tag="t")
                    nc.tensor.transpose(pt[:], xt[:, ko * P : (ko + 1) * P], ident_r[:])
                    nc.vector.tensor_scalar_mul(
                        xT[:, ko, r * P : (r + 1) * P], pt[:], 2048.0
                    )

        # Phase 1 block: v'^T[qkv, rows n-slice] for one n; bias in copyback
        def gemm1(n):
            for m in range(MQ):
                ps = mmp.tile([P, NT], f32, tag="g1")
                for ko in range(KO):
                    mm(
                        ps[:],
                        wv_sb[:, ko, m * P : (m + 1) * P],
                        xT[:, ko, n * NT : (n + 1) * NT],
                        start=(ko == 0),
                        stop=(ko == KO - 1),
                    )
                nc.vector.tensor_scalar_add(
                    vT[:, m, n * NT : (n + 1) * NT], ps[:], bv2[:, m : m + 1]
                )

        # Phase 2 block: out row-tile m; LDWEIGHTS shared across the n-pair
        # (lhsT = vT tile stays stationary for both 512-wide output halves)
        def gemm2(m):
            pss = [mmp.tile([P, NT], f32, tag="g2", name=f"g2_{m}_{n}") for n in range(2)]
            for ko in range(MQ):
                for n in range(2):
                    mm(
                        pss[n][:],
                        vT[:, ko, m * P : (m + 1) * P],
                        wo_sb[:, ko, n * NT : (n + 1) * NT],
                        start=(ko == 0),
                        stop=(ko == MQ - 1),
                    )
            for n in range(2):
                ot = outp.tile([P, NT], f32)
                nc.vector.tensor_tensor(
                    ot[:], pss[n][:], bo_rep[:, n * NT : (n + 1) * NT],
                    mybir.AluOpType.add,
                )
                nc.sync.dma_start(
                    out_nd[m * P : (m + 1) * P, n * NT : (n + 1) * NT], ot[:]
                )

        transpose_rows(range(0, 4))
        gemm1(0)
        transpose_rows(range(4, RT))
        gemm1(1)
        for m in range(RT):
            gemm2(m)

    nc.compile()
    _NC_CACHE["nc"] = nc
    return nc


def make_in_maps(inputs):
    xf = np.ascontiguousarray(
        np.asarray(inputs["x"], dtype=np.float32).reshape(ROWS, D)
    )
    wv = np.ascontiguousarray(np.asarray(inputs["Wv"], dtype=np.float32))
    bv = np.ascontiguousarray(np.asarray(inputs["bv"], dtype=np.float32))
    wo = np.ascontiguousarray(np.asarray(inputs["Wo"], dtype=np.float32))
    bo = np.ascontiguousarray(np.asarray(inputs["bo"], dtype=np.float32))
    return [
        {
            "x": xf[c * R : (c + 1) * R],
            "wv": wv,
            "bv": bv,
            "wo": wo,
            "bo": bo,
        }
        for c in range(NCORES)
    ]


def kernel(**inputs) -> np.ndarray:
    from concourse.bass_utils import run_bass_kernel_spmd

    nc = build_nc()
    in_maps = make_in_maps(inputs)
    res = run_bass_kernel_spmd(nc, in_maps, list(range(NCORES)))
    out = np.concatenate(
        [res.results[c]["out"] for c in range(NCORES)], axis=0
    ).reshape(B, L, D)
    return np.ascontiguousarray(out.astype(np.float32, copy=False))


# revision 18
# speedup vs baseline: 3.1073x; 1.0008x over previous
"""Trainium2 Bass kernel for nn_CrossAttention_43061342110469.

Mathematical reduction: the reference's second einsum
    attn = einsum('bvhd,bhqk->bvhd', v, scores)
shares no contraction index with v, so it multiplies v elementwise by
S[b,h] = sum_{q,k} scores[b,h,q,k].  scores is a softmax over k, so every
row sums to 1 and S[b,h] == L == 2048 (exactly, even in fp32 — verified:
the fp32 reference computes S == 2048.0 bit-exactly, and the end-to-end
rel-err of this reduction vs the reference is ~5e-7).

Therefore:
    out = (x @ Wv + bv) * 2048 @ Wo + bo
        = ((2048*x) @ Wv + 2048*bv) @ Wo + bo

Kernel: row-shard the flattened [8192, 1024] x across 8 cores (1024 rows
each); each core runs two chained 1024x1024x1024 fp32 GEMMs:
    Phase T: DMA x rows, PE-transpose 128x128 tiles (scaling by 2048) into
             an SBUF x^T buffer (fp32 has no DMA-transpose path).
    Phase 1: v'^T[qkv, row] = Wv^T-tiles.T-free GEMM accumulating over d,
             plus a K=1 outer-product matmul adding 2048*bv.
    Phase 2: out[row, d_out] = v'^T-tiles as lhsT against Wo, plus a K=1
             outer-product matmul adding bo; DMA result tiles out.
q/k/softmax are numerically dead and not computed.
"""

import sys

import numpy as np

_REPO = "/opt/trn_rl_repo"
if _REPO not in sys.path:
    sys.path.insert(0, _REPO)

B, L, D = 4, 2048, 1024
NQKV = 1024  # QKV * H = 64 * 16
NCORES = 8
ROWS = B * L  # 8192
R = ROWS // NCORES  # 1024 rows per core
P = 128
NT = 512  # matmul free-dim tile (one PSUM bank of fp32)

# "float32" = exact fp32 matmul (2 HW passes, 4 cyc/row).
# "float32r" = single-pass fp32 matmul (1 cyc/row at N>=512), reduced
# internal mantissa; precision validated end-to-end against the reference.
MM_DTYPE = "float32r"

_NC_CACHE = {}


def build_nc():
    """Build + compile the per-core Bass program (cached)."""
    if "nc" in _NC_CACHE:
        return _NC_CACHE["nc"]

    from contextlib import ExitStack

    import concourse.tile as tile
    from concourse import bacc, mybir
    from concourse._compat import get_trn_type
    from concourse.masks import make_identity

    f32 = mybir.dt.float32
    mmdt = getattr(mybir.dt, MM_DTYPE)

    def mm(ps, lhsT, rhs, start, stop):
        nc.tensor.matmul(ps, lhsT=lhsT, rhs=rhs, start=start, stop=stop)

    nc = bacc.Bacc(
        get_trn_type() or "TRN2",
        target_bir_lowering=False,
        debug=False,
        num_devices=NCORES,
    )

    x_nd = nc.dram_tensor("x", [R, D], f32, kind="ExternalInput").ap()
    wv_nd = nc.dram_tensor("wv", [D, NQKV], f32, kind="ExternalInput").ap()
    bv_nd = nc.dram_tensor("bv", [NQKV], f32, kind="ExternalInput").ap()
    wo_nd = nc.dram_tensor("wo", [NQKV, D], f32, kind="ExternalInput").ap()
    bo_nd = nc.dram_tensor("bo", [D], f32, kind="ExternalInput").ap()
    out_nd = nc.dram_tensor("out", [R, D], f32, kind="ExternalOutput").ap()

    KO = D // P  # 8 contraction tiles for GEMM1
    MQ = NQKV // P  # 8 qkv tiles (contraction tiles for GEMM2)
    RT = R // P  # 8 row tiles

    with tile.TileContext(nc) as tc, ExitStack() as ctx:
        const = ctx.enter_context(tc.tile_pool(name="const", bufs=1))
        big = ctx.enter_context(tc.tile_pool(name="big", bufs=1))
        xrow = ctx.enter_context(tc.tile_pool(name="xrow", bufs=8))
        # one PSUM pool, one tag, 8 slots = all 8 banks; transposes cycle
        # through it and GEMM1's ko-outer form holds all 8 at once
        psp = ctx.enter_context(tc.tile_pool(name="psp", bufs=8, space="PSUM"))
        outp = ctx.enter_context(tc.tile_pool(name="outp", bufs=4))

        # --- PE warmup: ~4us of dummy matmuls so the HAM clock-gate opens
        # (K=8/8 @ 2.4GHz) before the real transposes arrive.
        warm = const.tile([P, P], f32)
        nc.vector.memset(warm[:], 1.0)
        wps = psp.tile([P, P], f32, tag="t", name="wps")
        for _ in range(12):
            nc.tensor.matmul(wps[:], lhsT=warm[:], rhs=warm[:], start=True, stop=True)

        ident = const.tile([P, P], f32)
        make_identity(nc, ident)
        ident_r = const.tile([P, P], mmdt)
        nc.vector.tensor_copy(ident_r[:], ident[:])

        # bv2[p, o] = 2048*bv[o*128+p]: per-partition scalar for the GEMM1
        # copyback (tiny, early on scalar queue)
        bv2 = const.tile([P, NQKV // P], f32)
        nc.scalar.dma_start(bv2[:], bv_nd.rearrange("(o p) -> p o", p=P))
        nc.vector.tensor_scalar_mul(bv2[:], bv2[:], 2048.0)

        wv_sb = big.tile([P, KO, NQKV], mmdt)
        wo_sb = big.tile([P, MQ, D], mmdt)
        xT = big.tile([P, KO, R], mmdt)  # [d_inner, d_outer, row] = (2048*x)^T
        vT = big.tile([P, MQ, R], mmdt)  # [qkv_inner, qkv_outer, row]

        # DMA priority schedule, round-robin over the 3 issue queues (each
        # dma_start packet-shards over all 16 DMA engines; concurrency across
        # queues buys aggregate HBM BW):
        #   x rows 0-3 (feed transposes ~11us) -> Wv chunks (GEMM1's ko-outer
        #   loop consumes chunk ko as it lands) -> x rows 4-7 -> Wo spread ->
        #   bo_rep (only needed by GEMM2 copybacks ~45us+).
        wv_r = wv_nd.rearrange("(ko p) n -> p ko n", p=P).bitcast(mmdt)
        wo_r = wo_nd.rearrange("(ko p) n -> p ko n", p=P).bitcast(mmdt)
        xts = []
        for r in range(RT):
            xts.append(xrow.tile([P, D], mmdt, tag="xt", name=f"xt{r}"))
        jobs = []
        for r in range(4):
            jobs.append((xts[r][:], x_nd[r * P : (r + 1) * P, :].bitcast(mmdt)))
        for ko in range(KO):
            jobs.append((wv_sb[:, ko], wv_r[:, ko]))
        for r in range(4, RT):
            jobs.append((xts[r][:], x_nd[r * P : (r + 1) * P, :].bitcast(mmdt)))
        for ko in range(MQ):
            jobs.append((wo_sb[:, ko], wo_r[:, ko]))
        qs = [nc.sync, nc.scalar, nc.gpsimd]
        for i, (dst, srcap) in enumerate(jobs):
            qs[i % 3].dma_start(dst, srcap)
        bo_rep = const.tile([P, D], f32)
        nc.gpsimd.dma_start(bo_rep[:], bo_nd[None, :].to_broadcast((P, D)))

        # Phase T: transpose 128x128 blocks on PE (f32r path), scale by 2048
        def transpose_rows(rs):
            for r in rs:
                xt = xts[r]
                for ko in range(KO):
                    pt = psp.tile([P, P], mmdt, tag="t", name=f"pt{r}_{ko}")
                    nc.tensor.transpose(pt[:], xt[:, ko * P : (ko + 1) * P], ident_r[:])
                    nc.vector.tensor_scalar_mul(
                        xT[:, ko, r * P : (r + 1) * P], pt[:], 2048.0
                    )

        # GEMM1 block for one 512-row slice, ko-outer: all 8 qkv-tile psums
        # live at once, so contraction step ko can run as soon as Wv chunk ko
        # lands (no wait for the full Wv)
        def gemm1(n):
            pss = [psp.tile([P, NT], f32, tag="t", name=f"g1_{n}_{m}") for m in range(MQ)]
            for ko in range(KO):
                for m in range(MQ):
                    mm(
                        pss[m][:],
                        wv_sb[:, ko, m * P : (m + 1) * P],
                        xT[:, ko, n * NT : (n + 1) * NT],
                        start=(ko == 0),
                        stop=(ko == KO - 1),
                    )
            for m in range(MQ):
                nc.vector.tensor_scalar_add(
                    vT[:, m, n * NT : (n + 1) * NT], pss[m][:], bv2[:, m : m + 1]
                )

        # GEMM2: out row-tile m, output halves n; out DMAs round-robin the
        # three issue queues so the drain keeps pace with production
        def gemm2(m):
            for n in range(2):
                ps = psp.tile([P, NT], f32, tag="t", name=f"g2_{m}_{n}")
                for ko in range(MQ):
                    mm(
                        ps[:],
                        vT[:, ko, m * P : (m + 1) * P],
                        wo_sb[:, ko, n * NT : (n + 1) * NT],
                        start=(ko == 0),
                        stop=(ko == MQ - 1),
                    )
                ot = outp.tile([P, NT], f32)
                nc.vector.tensor_tensor(
                    ot[:], ps[:], bo_rep[:, n * NT : (n + 1) * NT],
                    mybir.AluOpType.add,
                )
                qs[(2 * m + n) % 3].dma_start(
                    out_nd[m * P : (m + 1) * P, n * NT : (n + 1) * NT], ot[:]
                )

        transpose_rows(range(0, 4))
        gemm1(0)
        transpose_rows(range(4, RT))
        gemm1(1)
        for m in range(RT):
            gemm2(m)

    nc.compile()
    _NC_CACHE["nc"] = nc
    return nc


def make_in_maps(inputs):
    xf = np.ascontiguousarray(
        np.asarray(inputs["x"], dtype=np.float32).reshape(ROWS, D)
    )
    wv = np.ascontiguousarray(np.asarray(inputs["Wv"], dtype=np.float32))
    bv = np.ascontiguousarray(np.asarray(inputs["bv"], dtype=np.float32))
    wo = np.ascontiguousarray(np.asarray(inputs["Wo"], dtype=np.float32))
    bo = np.ascontiguousarray(np.asarray(inputs["bo"], dtype=np.float32))
    return [
        {
            "x": xf[c * R : (c + 1) * R],
            "wv": wv,
            "bv": bv,
            "wo": wo,
            "bo": bo,
        }
        for c in range(NCORES)
    ]


def kernel(**inputs) -> np.ndarray:
    from concourse.bass_utils import run_bass_kernel_spmd

    nc = build_nc()
    in_maps = make_in_maps(inputs)
    res = run_bass_kernel_spmd(nc, in_maps, list(range(NCORES)))
    out = np.concatenate(
        [res.results[c]["out"] for c in range(NCORES)], axis=0
    ).reshape(B, L, D)
    return np.ascontiguousarray(out.astype(np.float32, copy=False))


# revision 19
# speedup vs baseline: 3.2139x; 1.0343x over previous
"""Trainium2 Bass kernel for nn_CrossAttention_43061342110469.

Mathematical reduction: the reference's second einsum
    attn = einsum('bvhd,bhqk->bvhd', v, scores)
shares no contraction index with v, so it multiplies v elementwise by
S[b,h] = sum_{q,k} scores[b,h,q,k].  scores is a softmax over k, so every
row sums to 1 and S[b,h] == L == 2048 (exactly, even in fp32 — verified:
the fp32 reference computes S == 2048.0 bit-exactly, and the end-to-end
rel-err of this reduction vs the reference is ~5e-7).

Therefore:
    out = (x @ Wv + bv) * 2048 @ Wo + bo
        = ((2048*x) @ Wv + 2048*bv) @ Wo + bo

Kernel: row-shard the flattened [8192, 1024] x across 8 cores (1024 rows
each); each core runs two chained 1024x1024x1024 fp32 GEMMs:
    Phase T: DMA x rows, PE-transpose 128x128 tiles (scaling by 2048) into
             an SBUF x^T buffer (fp32 has no DMA-transpose path).
    Phase 1: v'^T[qkv, row] = Wv^T-tiles.T-free GEMM accumulating over d,
             plus a K=1 outer-product matmul adding 2048*bv.
    Phase 2: out[row, d_out] = v'^T-tiles as lhsT against Wo, plus a K=1
             outer-product matmul adding bo; DMA result tiles out.
q/k/softmax are numerically dead and not computed.
"""

import sys

import numpy as np

_REPO = "/opt/trn_rl_repo"
if _REPO not in sys.path:
    sys.path.insert(0, _REPO)

B, L, D = 4, 2048, 1024
NQKV = 1024  # QKV * H = 64 * 16
NCORES = 8
ROWS = B * L  # 8192
R = ROWS // NCORES  # 1024 rows per core
P = 128
NT = 512  # matmul free-dim tile (one PSUM bank of fp32)

# "float32" = exact fp32 matmul (2 HW passes, 4 cyc/row).
# "float32r" = single-pass fp32 matmul (1 cyc/row at N>=512), reduced
# internal mantissa; precision validated end-to-end against the reference.
MM_DTYPE = "float32r"

_NC_CACHE = {}


def build_nc():
    """Build + compile the per-core Bass program (cached)."""
    if "nc" in _NC_CACHE:
        return _NC_CACHE["nc"]

    from contextlib import ExitStack

    import concourse.tile as tile
    from concourse import bacc, mybir
    from concourse.tile_rust import add_dep_helper
    from concourse._compat import get_trn_type
    from concourse.masks import make_identity

    f32 = mybir.dt.float32
    mmdt = getattr(mybir.dt, MM_DTYPE)

    def mm(ps, lhsT, rhs, start, stop):
        nc.tensor.matmul(ps, lhsT=lhsT, rhs=rhs, start=start, stop=stop)

    nc = bacc.Bacc(
        get_trn_type() or "TRN2",
        target_bir_lowering=False,
        debug=False,
        num_devices=NCORES,
    )

    x_nd = nc.dram_tensor("x", [R, D], f32, kind="ExternalInput").ap()
    wv_nd = nc.dram_tensor("wv", [D, NQKV], f32, kind="ExternalInput").ap()
    bv_nd = nc.dram_tensor("bv", [NQKV], f32, kind="ExternalInput").ap()
    wo_nd = nc.dram_tensor("wo", [NQKV, D], f32, kind="ExternalInput").ap()
    bo_nd = nc.dram_tensor("bo", [D], f32, kind="ExternalInput").ap()
    out_nd = nc.dram_tensor("out", [R, D], f32, kind="ExternalOutput").ap()

    KO = D // P  # 8 contraction tiles for GEMM1
    MQ = NQKV // P  # 8 qkv tiles (contraction tiles for GEMM2)
    RT = R // P  # 8 row tiles

    with tile.TileContext(nc) as tc, ExitStack() as ctx:
        const = ctx.enter_context(tc.tile_pool(name="const", bufs=1))
        big = ctx.enter_context(tc.tile_pool(name="big", bufs=1))
        xrow = ctx.enter_context(tc.tile_pool(name="xrow", bufs=8))
        # one PSUM pool, one tag, 8 slots = all 8 banks; transposes cycle
        # through it and GEMM1's ko-outer form holds all 8 at once
        psp = ctx.enter_context(tc.tile_pool(name="psp", bufs=8, space="PSUM"))
        outp = ctx.enter_context(tc.tile_pool(name="outp", bufs=4))

        # --- PE warmup: ~4us of dummy matmuls so the HAM clock-gate opens
        # (K=8/8 @ 2.4GHz) before the real transposes arrive.
        warm = const.tile([P, P], f32)
        nc.vector.memset(warm[:], 1.0)
        wps = psp.tile([P, P], f32, tag="t", name="wps")
        for _ in range(12):
            nc.tensor.matmul(wps[:], lhsT=warm[:], rhs=warm[:], start=True, stop=True)

        ident = const.tile([P, P], f32)
        make_identity(nc, ident)
        ident_r = const.tile([P, P], mmdt)
        nc.vector.tensor_copy(ident_r[:], ident[:])

        # bv2[p, o] = 2048*bv[o*128+p]: per-partition scalar for the GEMM1
        # copyback (tiny, early on scalar queue)
        bv2 = const.tile([P, NQKV // P], f32)
        nc.scalar.dma_start(bv2[:], bv_nd.rearrange("(o p) -> p o", p=P))
        nc.vector.tensor_scalar_mul(bv2[:], bv2[:], 2048.0)

        wv_sb = big.tile([P, KO, NQKV], mmdt)
        wo_sb = big.tile([P, MQ, D], mmdt)
        xT = big.tile([P, KO, R], mmdt)  # [d_inner, d_outer, row] = (2048*x)^T
        vT = big.tile([P, MQ, R], mmdt)  # [qkv_inner, qkv_outer, row]

        # DMA priority schedule, round-robin over the 3 issue queues (each
        # dma_start packet-shards over all 16 DMA engines; concurrency across
        # queues buys aggregate HBM BW):
        #   x rows 0-3 (feed transposes ~11us) -> Wv chunks (GEMM1's ko-outer
        #   loop consumes chunk ko as it lands) -> x rows 4-7 -> Wo spread ->
        #   bo_rep (only needed by GEMM2 copybacks ~45us+).
        wv_r = wv_nd.rearrange("(ko p) n -> p ko n", p=P).bitcast(mmdt)
        wo_r = wo_nd.rearrange("(ko p) n -> p ko n", p=P).bitcast(mmdt)
        xts = []
        for r in range(RT):
            xts.append(xrow.tile([P, D], mmdt, tag="xt", name=f"xt{r}"))
        jobs = []
        for r in range(4):
            jobs.append((xts[r][:], x_nd[r * P : (r + 1) * P, :].bitcast(mmdt)))
        for ko in range(KO):
            jobs.append((wv_sb[:, ko], wv_r[:, ko]))
        for r in range(4, RT):
            jobs.append((xts[r][:], x_nd[r * P : (r + 1) * P, :].bitcast(mmdt)))
        for ko in range(MQ):
            jobs.append((wo_sb[:, ko], wo_r[:, ko]))
        # Depth-2 completion chains per issue queue: without them every
        # dma_start floods the shared 16-engine fabric at once and each
        # transfer's completion latency becomes (all bytes ahead of it)/BW —
        # the first x row then lands at ~20us instead of ~11us.
        qs = [nc.sync, nc.scalar, nc.gpsimd]
        chains = [[], [], []]

        def chained_dma(qi, dst, srcap):
            inst = qs[qi].dma_start(dst, srcap)
            ch = chains[qi]
            if len(ch) >= 2:
                add_dep_helper(inst.ins, ch[-2].ins, sync=True, reason="dma chain")
            ch.append(inst)
            return inst

        for i, (dst, srcap) in enumerate(jobs):
            chained_dma(i % 3, dst, srcap)
        bo_rep = const.tile([P, D], f32)
        chained_dma(2, bo_rep[:], bo_nd[None, :].to_broadcast((P, D)))

        # Phase T: transpose 128x128 blocks on PE (f32r path), scale by 2048
        def transpose_rows(rs):
            for r in rs:
                xt = xts[r]
                for ko in range(KO):
                    pt = psp.tile([P, P], mmdt, tag="t", name=f"pt{r}_{ko}")
                    nc.tensor.transpose(pt[:], xt[:, ko * P : (ko + 1) * P], ident_r[:])
                    nc.vector.tensor_scalar_mul(
                        xT[:, ko, r * P : (r + 1) * P], pt[:], 2048.0
                    )

        # GEMM1 block for one 512-row slice, ko-outer: all 8 qkv-tile psums
        # live at once, so contraction step ko can run as soon as Wv chunk ko
        # lands (no wait for the full Wv)
        def gemm1(n):
            pss = [psp.tile([P, NT], f32, tag="t", name=f"g1_{n}_{m}") for m in range(MQ)]
            for ko in range(KO):
                for m in range(MQ):
                    mm(
                        pss[m][:],
                        wv_sb[:, ko, m * P : (m + 1) * P],
                        xT[:, ko, n * NT : (n + 1) * NT],
                        start=(ko == 0),
                        stop=(ko == KO - 1),
                    )
            for m in range(MQ):
                nc.vector.tensor_scalar_add(
                    vT[:, m, n * NT : (n + 1) * NT], pss[m][:], bv2[:, m : m + 1]
                )

        # GEMM2: out row-tile m, output halves n; out DMAs round-robin the
        # three issue queues so the drain keeps pace with production
        def gemm2(m):
            for n in range(2):
                ps = psp.tile([P, NT], f32, tag="t", name=f"g2_{m}_{n}")
                for ko in range(MQ):
                    mm(
                        ps[:],
                        vT[:, ko, m * P : (m + 1) * P],
                        wo_sb[:, ko, n * NT : (n + 1) * NT],
                        start=(ko == 0),
                        stop=(ko == MQ - 1),
                    )
                ot = outp.tile([P, NT], f32)
                nc.vector.tensor_tensor(
                    ot[:], ps[:], bo_rep[:, n * NT : (n + 1) * NT],
                    mybir.AluOpType.add,
                )
                chained_dma(
                    (2 * m + n) % 3,
                    out_nd[m * P : (m + 1) * P, n * NT : (n + 1) * NT],
                    ot[:],
                )

        transpose_rows(range(0, 4))
        gemm1(0)
        transpose_rows(range(4, RT))
        gemm1(1)
        for m in range(RT):
            gemm2(m)

    nc.compile()
    _NC_CACHE["nc"] = nc
    return nc


def make_in_maps(inputs):
    xf = np.ascontiguousarray(
        np.asarray(inputs["x"], dtype=np.float32).reshape(ROWS, D)
    )
    wv = np.ascontiguousarray(np.asarray(inputs["Wv"], dtype=np.float32))
    bv = np.ascontiguousarray(np.asarray(inputs["bv"], dtype=np.float32))
    wo = np.ascontiguousarray(np.asarray(inputs["Wo"], dtype=np.float32))
    bo = np.ascontiguousarray(np.asarray(inputs["bo"], dtype=np.float32))
    return [
        {
            "x": xf[c * R : (c + 1) * R],
            "wv": wv,
            "bv": bv,
            "wo": wo,
            "bo": bo,
        }
        for c in range(NCORES)
    ]


def kernel(**inputs) -> np.ndarray:
    from concourse.bass_utils import run_bass_kernel_spmd

    nc = build_nc()
    in_maps = make_in_maps(inputs)
    res = run_bass_kernel_spmd(nc, in_maps, list(range(NCORES)))
    out = np.concatenate(
        [res.results[c]["out"] for c in range(NCORES)], axis=0
    ).reshape(B, L, D)
    return np.ascontiguousarray(out.astype(np.float32, copy=False))
